# revision 13
# baseline (speedup 1.0000x reference)
"""Trainium2 Bass kernel for nn_Model_26439818674684.

Architecture (from the reference):
  - text LSTM over 600=30*20 sequences of len 128 (E=300 -> H=256). Only
    LAYER 0's final hidden state is consumed downstream, so only layer 0 is
    computed.
  - topic LSTM (2 layers, batch=30 days, T=20 topics, H=256)
  - per-day attention with a sorted-cumsum keep mask (sort-free via pairwise
    comparisons)
  - day LSTM (2 layers, batch=1, T=30, 256 -> 64), small attention + head.

Key structure vs the v0 kernel:
  - Sequences are sharded TOPIC-MAJOR (600 cols ordered tp*30+d) so the topic
    LSTM's per-step batch is a contiguous 30-column slice.
  - Text LSTM: 75 seqs/core padded to 76 = 2 groups of 38, software-pipelined
    so one group's matmuls overlap the other group's activation chain.
  - Input gates accumulate directly in PSUM; Whh recurrence matmuls accumulate
    on top (start=False). No identity-seed matmuls, no PSUM->SBUF gate copies.
  - Single-sigmoid trick: g-gate rows pre-scaled by 2, tanh(g) = 2*sig(2g)-1.
    One Sigmoid activation covers all 4 gates.
  - LSTM weights in fp8e4m3 scaled by 16 (sigmoid activation applies 1/16),
    activations bf16. Validated host-side: final rel err ~6e-5 (tol 2e-2).
  - Day LSTM: both layers fused into one 128-partition instruction stream
    (h0 on partitions 0:64, h1 on 64:128), halving the serial round count.
"""
import sys
sys.path.insert(0, '/opt/trn_rl_repo')

import numpy as np
import ml_dtypes

import concourse.bass as bass
import concourse.tile as tile
from concourse import bacc, mybir
from concourse.bass_utils import run_bass_kernel_spmd

F32 = mybir.dt.float32
BF16 = mybir.dt.bfloat16
F8 = mybir.dt.float8e4
AF = mybir.ActivationFunctionType
ALU = mybir.AluOpType
BF = ml_dtypes.bfloat16
F8H = ml_dtypes.float8_e4m3fn

NC_ = 8
DAYS, TOPICS, T, E, H, DH = 30, 20, 128, 300, 256, 64
B = DAYS * TOPICS          # 600
BC = B // NC_              # 75 sequences per core
GB = 38                    # group batch (75 padded to 76 = 2*38)
BCP = 2 * GB
EP = 384                   # E padded to 3 K-tiles (row 300 = bias ones-row)
WS = 16.0                  # fp8 weight scale (sigmoid applies 1/WS)
ISC = 1.0 / WS
CH = 4                     # topic L0 input-gate chunk (steps)

_cache = {}


def build():
    nc = bacc.Bacc("TRN2", target_bir_lowering=False, debug=False,
                   enable_asserts=False, num_devices=NC_)

    # ---------------- DRAM I/O ----------------
    x_d = nc.dram_tensor("x", [T // 8, 128, 8, 3, BCP], BF16,
                         kind="ExternalInput")
    wih0_d = nc.dram_tensor("wih0", [EP, 4 * H], F8, kind="ExternalInput")
    whh0_d = nc.dram_tensor("whh0", [H, 4 * H], F8, kind="ExternalInput")
    ident_d = nc.dram_tensor("ident", [128, 128], BF16, kind="ExternalInput")
    identf_d = nc.dram_tensor("identf", [32, 32], F32, kind="ExternalInput")
    ones_p_d = nc.dram_tensor("ones_p", [128, 1], BF16, kind="ExternalInput")
    ones_f_d = nc.dram_tensor("ones_f", [1, 128], BF16, kind="ExternalInput")
    ones6_d = nc.dram_tensor("ones6", [1, B], BF16, kind="ExternalInput")
    ones_f32_d = nc.dram_tensor("ones_f32", [1, 64], F32, kind="ExternalInput")
    t_wih0_d = nc.dram_tensor("t_wih0", [H, 4 * H], F8, kind="ExternalInput")
    t_whh0_d = nc.dram_tensor("t_whh0", [H, 4 * H], F8, kind="ExternalInput")
    t_wih1_d = nc.dram_tensor("t_wih1", [H, 4 * H], F8, kind="ExternalInput")
    t_whh1_d = nc.dram_tensor("t_whh1", [H, 4 * H], F8, kind="ExternalInput")
    t_b0r_d = nc.dram_tensor("t_b0r", [1, 4 * H], BF16, kind="ExternalInput")
    t_b1bc_d = nc.dram_tensor("t_b1bc", [128, 8, DAYS], BF16,
                              kind="ExternalInput")
    w1t_d = nc.dram_tensor("w1t", [H, H], BF16, kind="ExternalInput")
    w1b_d = nc.dram_tensor("w1b", [128, 2], F32, kind="ExternalInput")
    d_wih0_d = nc.dram_tensor("d_wih0", [H, 4, DH], F8, kind="ExternalInput")
    d_wc_d = nc.dram_tensor("d_wc", [128, 4, 128], F8, kind="ExternalInput")
    d_seed_d = nc.dram_tensor("d_seed", [128, 4, DAYS], BF16,
                              kind="ExternalInput")
    w2t_d = nc.dram_tensor("w2t", [DH, DH], F32, kind="ExternalInput")
    w2b_d = nc.dram_tensor("w2b", [DH, 1], F32, kind="ExternalInput")
    l1t_d = nc.dram_tensor("l1t", [DH, 48], F32, kind="ExternalInput")
    l1b_d = nc.dram_tensor("l1b", [48, 1], F32, kind="ExternalInput")
    l2t_d = nc.dram_tensor("l2t", [48, 16], F32, kind="ExternalInput")
    l2b_d = nc.dram_tensor("l2b", [16, 1], F32, kind="ExternalInput")
    hw16_d = nc.dram_tensor("hw16", [16, 4], F32, kind="ExternalInput")
    hw4_d = nc.dram_tensor("hw4", [4, 4], F32, kind="ExternalInput")
    hb_d = nc.dram_tensor("hb", [4, 1], F32, kind="ExternalInput")
    prev_d = nc.dram_tensor("prev", [4, 4], F32, kind="ExternalInput")
    res_d = nc.dram_tensor("res", [4, 1], F32, kind="ExternalOutput")

    with tile.TileContext(nc) as tc:
        with tc.tile_pool(name="persist", bufs=1) as pp, \
             tc.tile_pool(name="act", bufs=4) as ap_, \
             tc.tile_pool(name="dram", bufs=1, space="DRAM") as dp:

            # ======== Phase A: text LSTM layer 0, 2x38 pipelined groups ====
            wih = pp.tile([128, 3, 4 * H], F8, tag="wih")
            nc.sync.dma_start(wih[:], wih0_d.ap().rearrange("(k p) m -> p k m", p=128))
            whh = pp.tile([128, 2, 4 * H], F8, tag="whh")
            nc.sync.dma_start(whh[:], whh0_d.ap().rearrange("(j p) m -> p j m", p=128))

            hb_t = pp.tile([128, 2, BCP], BF16, tag="hb_t")
            cb_t = pp.tile([128, 2, BCP], BF16, tag="cb_t")
            nc.any.memset(cb_t[:], 0.0)

            ctxA = nc.named_scope("phaseA_text")
            ctxA.__enter__()
            with tc.tile_pool(name="xin", bufs=3) as xip, \
                 tc.tile_pool(name="gps", bufs=4, space="PSUM") as gps:
                xts = {}

                def rec_mms(gp, m_lo, m_hi):
                    for m in range(m_lo, m_hi):
                        for j in range(2):
                            nc.tensor.matmul(
                                gp[:, m // 4, m % 4, 0:BCP],
                                whh[:, j, 128 * m:128 * (m + 1)],
                                hb_t[:, j, :], start=False, stop=(j == 1),
                                skip_group_check=True)

                for t in range(T):
                    if t % 8 == 0:
                        xt = xip.tile([128, 8, 3, BCP], BF16, tag="xt")
                        nc.sync.dma_start(xt[:], x_d.ap()[t // 8])
                        xts[t // 8] = xt
                    xt = xts[t // 8]
                    gp = gps.tile([128, 2, 4, 128], F32, tag="g", name="gp")
                    for m in range(8):
                        dst = gp[:, m // 4, m % 4, 0:BCP]
                        for k in range(3):
                            nc.tensor.matmul(
                                dst, wih[:, k, 128 * m:128 * (m + 1)],
                                xt[:, t % 8, k, :], start=(k == 0),
                                stop=(k == 2 and t == 0),
                                skip_group_check=True)
                    # recurrence accumulates on top of the input gates.
                    # m-tiles 0:6 = i,f,g gates; 6:8 = o gates. The i/f/g
                    # sigmoid fires as soon as the first 12 recurrence MMs
                    # land; the o-gate MMs + sigmoid overlap the vector chain.
                    if t > 0:
                        rec_mms(gp, 0, 6)
                    gpm = gp.rearrange("p a m x -> p (a m) x")
                    s6 = ap_.tile([128, 6, BCP], BF16, tag="s6", name="s6")
                    nc.scalar.activation(s6[:], gpm[:, 0:6, 0:BCP], AF.Sigmoid,
                                         scale=ISC)
                    if t > 0:
                        rec_mms(gp, 6, 8)
                    s2 = ap_.tile([128, 2, BCP], BF16, tag="s2", name="s2")
                    nc.scalar.activation(s2[:], gpm[:, 6:8, 0:BCP], AF.Sigmoid,
                                         scale=ISC)
                    tg = ap_.tile([128, 2, BCP], BF16, tag="tg", name="tg")
                    nc.vector.tensor_scalar(tg[:], s6[:, 4:6], 2.0, -1.0,
                                            op0=ALU.mult, op1=ALU.add)
                    u = ap_.tile([128, 2, BCP], BF16, tag="u", name="u")
                    nc.vector.tensor_mul(u[:], s6[:, 0:2], tg[:])
                    v = ap_.tile([128, 2, BCP], BF16, tag="v", name="v")
                    nc.vector.tensor_mul(v[:], s6[:, 2:4], cb_t[:])
                    nc.vector.tensor_add(cb_t[:], u[:], v[:])
                    tc_ = ap_.tile([128, 2, BCP], BF16, tag="tc", name="tc")
                    nc.scalar.activation(tc_[:], cb_t[:], AF.Tanh)
                    nc.vector.tensor_mul(hb_t[:], s2[:], tc_[:])

            ctxA.__exit__(None, None, None)
            # ======== Phase B: AllGather (topic-major columns) ========
            ctxB = nc.named_scope("phaseB_gather")
            ctxB.__enter__()
            hl = dp.tile([2, 128, BC], BF16, tag="hl")
            nc.sync.dma_start(hl[:],
                              hb_t[:, :, 0:BC].rearrange("p j b -> j p b"))
            gat = dp.tile([NC_, 2, 128, BC], BF16, tag="gat")
            nc.gpsimd.collective_compute(
                "AllGather", ALU.bypass,
                replica_groups=[list(range(NC_))],
                ins=[hl.opt()], outs=[gat.opt()])
            h_all = pp.tile([128, 2, B], BF16, tag="h_all")
            for r_ in range(NC_):
                nc.sync.dma_start(h_all[:, :, BC * r_:BC * (r_ + 1)],
                                  gat[r_].rearrange("j p b -> p j b"))
            ctxB.__exit__(None, None, None)

            # ======== Phase B2: topic LSTM (2 layers, 20 steps, N=30) ======
            ctxT = nc.named_scope("phaseB_topic")
            ctxT.__enter__()
            ones6 = pp.tile([1, B], BF16, tag="ones6")
            nc.sync.dma_start(ones6[:], ones6_d.ap())
            ones_f = pp.tile([1, 128], BF16, tag="ones_f")
            nc.sync.dma_start(ones_f[:], ones_f_d.ap())
            tw = {}
            for nm, d in (("t_wih0", t_wih0_d), ("t_whh0", t_whh0_d),
                          ("t_wih1", t_wih1_d), ("t_whh1", t_whh1_d)):
                tw[nm] = pp.tile([128, 2, 4 * H], F8, tag=nm, name=nm)
                nc.sync.dma_start(tw[nm][:],
                                  d.ap().rearrange("(j p) m -> p j m", p=128))
            b0r = pp.tile([1, 4 * H], BF16, tag="b0r")
            nc.sync.dma_start(b0r[:], t_b0r_d.ap())
            b1bc = pp.tile([128, 8, DAYS], BF16, tag="b1bc")
            nc.sync.dma_start(b1bc[:], t_b1bc_d.ap())

            y0 = pp.tile([128, 2, TOPICS, DAYS], BF16, tag="y0")
            ytop = pp.tile([128, 2, TOPICS, DAYS], BF16, tag="ytop")
            z30 = pp.tile([128, 2, DAYS], BF16, tag="z30")
            ct0 = pp.tile([128, 2, DAYS], BF16, tag="ct0")
            ct1 = pp.tile([128, 2, DAYS], BF16, tag="ct1")
            for ap0 in (z30, ct0, ct1):
                nc.any.memset(ap0[:], 0.0)

            with tc.tile_pool(name="tl0", bufs=2, space="PSUM") as tl0, \
                 tc.tile_pool(name="tl1", bufs=2, space="PSUM") as tl1:
                l0ch = {}

                def l0_chunk(c):
                    # input gates for CH steps: 120 contiguous tp-major cols
                    pc = tl0.tile([128, 2, 4, 128], F32, tag="pc",
                                  name="pc")
                    cs = slice(CH * DAYS * c, CH * DAYS * (c + 1))
                    for m in range(8):
                        dst = pc[:, m // 4, m % 4, 0:CH * DAYS]
                        for j in range(2):
                            nc.tensor.matmul(
                                dst, tw["t_wih0"][:, j, 128 * m:128 * (m + 1)],
                                h_all[:, j, cs], start=(j == 0), stop=False,
                                skip_group_check=True)
                        nc.tensor.matmul(
                            dst, b0r[0:1, 128 * m:128 * (m + 1)],
                            ones6[0:1, 0:CH * DAYS], start=False, stop=False,
                            skip_group_check=True)
                    l0ch[c] = pc

                def t_sig(gin, lyr):
                    s8 = ap_.tile([128, 8, DAYS], BF16, tag=f"ts8{lyr}",
                                  name="s8")
                    nc.scalar.activation(s8[:], gin, AF.Sigmoid, scale=ISC)
                    return s8

                def t_cell(s8, c_st, lyr):
                    tg = ap_.tile([128, 2, DAYS], BF16, tag=f"ttg{lyr}",
                                  name="tg")
                    nc.vector.tensor_scalar(tg[:], s8[:, 4:6], 2.0, -1.0,
                                            op0=ALU.mult, op1=ALU.add)
                    u = ap_.tile([128, 2, DAYS], BF16, tag=f"tu{lyr}", name="u")
                    nc.vector.tensor_mul(u[:], s8[:, 0:2], tg[:])
                    v = ap_.tile([128, 2, DAYS], BF16, tag=f"tv{lyr}", name="v")
                    nc.vector.tensor_mul(v[:], s8[:, 2:4], c_st[:])
                    nc.vector.tensor_add(c_st[:], u[:], v[:])

                def t_tanh(c_st, lyr):
                    tc_ = ap_.tile([128, 2, DAYS], BF16, tag=f"ttc{lyr}",
                                   name="tc")
                    nc.scalar.activation(tc_[:], c_st[:], AF.Tanh)
                    return tc_

                def l0_mms(t):
                    pc = l0ch[t // CH]
                    tl = t % CH
                    gs = slice(DAYS * tl, DAYS * (tl + 1))
                    hprev = z30 if t == 0 else y0[:, :, t - 1, :]
                    for m in range(8):
                        for j in range(2):
                            nc.tensor.matmul(
                                pc[:, m // 4, m % 4, gs],
                                tw["t_whh0"][:, j, 128 * m:128 * (m + 1)],
                                hprev[:, j, :], start=False, stop=(j == 1),
                                skip_group_check=True)
                    return pc.rearrange("p a m x -> p (a m) x")[:, :, gs]

                def l1_mms(t):
                    p1 = tl1.tile([128, 2, 4, 64], F32, tag="p1", name="p1")
                    hprev = z30 if t == 0 else ytop[:, :, t - 1, :]
                    for m in range(8):
                        dst = p1[:, m // 4, m % 4, 0:DAYS]
                        for j in range(2):
                            nc.tensor.matmul(
                                dst, tw["t_wih1"][:, j, 128 * m:128 * (m + 1)],
                                y0[:, j, t, :], start=(j == 0), stop=False,
                                skip_group_check=True)
                        for j in range(2):
                            nc.tensor.matmul(
                                dst, tw["t_whh1"][:, j, 128 * m:128 * (m + 1)],
                                hprev[:, j, :], start=False, stop=(j == 1),
                                skip_group_check=True)
                    g1 = ap_.tile([128, 8, DAYS], BF16, tag="g1", name="g1")
                    nc.vector.tensor_add(
                        g1[:], p1.rearrange("p a m x -> p (a m) x")[:, :, 0:DAYS],
                        b1bc[:])
                    return g1

                l0_chunk(0)
                l0_chunk(1)
                # steady state: L0 step t and L1 step t-1 pipelined with
                # engine streams interleaved (sig0, sig1, cell0, cell1, ...)
                g0in = l0_mms(0)
                s0 = t_sig(g0in, 0)
                t_cell(s0, ct0, 0)
                tc0 = t_tanh(ct0, 0)
                nc.vector.tensor_mul(y0[:, :, 0, :], s0[:, 6:8], tc0[:])
                for t in range(1, TOPICS):
                    if t % CH == 0 and t // CH + 1 < TOPICS // CH:
                        l0_chunk(t // CH + 1)
                    g0in = l0_mms(t)
                    s0 = t_sig(g0in, 0)
                    g1in = l1_mms(t - 1)
                    s1 = t_sig(g1in, 1)
                    t_cell(s0, ct0, 0)
                    t_cell(s1, ct1, 1)
                    tc0 = t_tanh(ct0, 0)
                    tc1 = t_tanh(ct1, 1)
                    nc.vector.tensor_mul(y0[:, :, t, :], s0[:, 6:8], tc0[:])
                    nc.vector.tensor_mul(ytop[:, :, t - 1, :], s1[:, 6:8],
                                         tc1[:])
                g1in = l1_mms(TOPICS - 1)
                s1 = t_sig(g1in, 1)
                t_cell(s1, ct1, 1)
                tc1 = t_tanh(ct1, 1)
                nc.vector.tensor_mul(ytop[:, :, TOPICS - 1, :], s1[:, 6:8],
                                     tc1[:])
            ctxT.__exit__(None, None, None)

            # ======== Phase C: topic attention (tp-major cols) ========
            ctxC = nc.named_scope("phaseC_attn")
            ctxC.__enter__()
            w1t = pp.tile([128, 2, H], BF16, tag="w1t")
            nc.sync.dma_start(w1t[:], w1t_d.ap().rearrange("(j p) m -> p j m", p=128))
            w1b = pp.tile([128, 2], F32, tag="w1b")
            nc.sync.dma_start(w1b[:], w1b_d.ap())
            ones_p = pp.tile([128, 1], BF16, tag="ones_p")
            nc.sync.dma_start(ones_p[:], ones_p_d.ap())
            ident = pp.tile([128, 128], BF16, tag="ident")
            nc.sync.dma_start(ident[:], ident_d.ap())
            identf = pp.tile([32, 32], F32, tag="identf")
            nc.sync.dma_start(identf[:], identf_d.ap())

            h_top = y0[:, :, TOPICS - 1, :]
            ytf = ytop.rearrange("p j t d -> p j (t d)")
            with tc.tile_pool(name="cps", bufs=2, space="PSUM") as cps, \
                 tc.tile_pool(name="mps", bufs=1, space="PSUM") as mps, \
                 tc.tile_pool(name="scps", bufs=1, space="PSUM") as scps:
                z = pp.tile([128, 2, B], F32, tag="z")
                for mi in range(2):
                    for nn in range(2):
                        cs = slice(300 * nn, 300 * (nn + 1))
                        pt = cps.tile([128, 300], F32, tag="zps")
                        for j in range(2):
                            nc.tensor.matmul(pt[:], w1t[:, j, 128 * mi:128 * (mi + 1)],
                                             ytf[:, j, cs], start=(j == 0), stop=(j == 1))
                        nc.scalar.activation(z[:, mi, cs], pt[:], AF.Identity,
                                             bias=w1b[:, mi:mi + 1])
                prod = pp.tile([128, 2, TOPICS, DAYS], BF16, tag="prod")
                z_r = z.rearrange("p j (tp d) -> p j tp d", tp=TOPICS)
                nc.vector.tensor_mul(
                    prod[:], z_r[:],
                    h_top.unsqueeze(2).broadcast_to([128, 2, TOPICS, DAYS]))
                prodf = prod.rearrange("p j tp d -> p j (tp d)")
                sc_ps = scps.tile([1, 2, 512], F32, tag="sc")
                for nn in range(2):
                    for j in range(2):
                        nc.tensor.matmul(sc_ps[0:1, nn, 0:300], ones_p[:, 0:1],
                                         prodf[:, j, 300 * nn:300 * (nn + 1)],
                                         start=(j == 0), stop=(j == 1))
                sc = pp.tile([1, B], F32, tag="sc_sb")
                nc.scalar.activation(sc.rearrange("p (nn x) -> p nn x", nn=2),
                                     sc_ps[0:1, :, 0:300], AF.Copy)
                # -> [30 days partitions, 20 topics] via DRAM + PE transpose
                d600 = dp.tile([B], F32, tag="d600")
                nc.sync.dma_start(d600[:], sc[0:1, :])
                sc20 = pp.tile([TOPICS, DAYS], F32, tag="sc20")
                nc.sync.dma_start(sc20[:], d600.rearrange("(tp d) -> tp d", tp=TOPICS))
                scT_ps = mps.tile([DAYS, TOPICS], F32, tag="scT")
                nc.tensor.transpose(scT_ps[0:DAYS, :], sc20[:], identf[0:TOPICS, 0:TOPICS])
                scT = pp.tile([DAYS, TOPICS], F32, tag="scT_sb")
                nc.vector.tensor_copy(scT[:], scT_ps[0:DAYS, :])
                # per-day softmax over topics (free dim)
                mx = pp.tile([DAYS, 1], F32, tag="mx")
                nc.vector.tensor_reduce(mx[:], scT[:], mybir.AxisListType.X, ALU.max)
                nmx = pp.tile([DAYS, 1], F32, tag="nmx")
                nc.scalar.mul(nmx[:], mx[:], -1.0)
                ex = pp.tile([DAYS, TOPICS], F32, tag="ex")
                nc.scalar.activation(ex[:], scT[:], AF.Exp, bias=nmx[:, 0:1])
                zs = pp.tile([DAYS, 1], F32, tag="zs")
                nc.vector.tensor_reduce(zs[:], ex[:], mybir.AxisListType.X, ALU.add)
                rz = pp.tile([DAYS, 1], F32, tag="rz")
                nc.vector.reciprocal(rz[:], zs[:])
                att_d = pp.tile([DAYS, TOPICS], F32, tag="att_d")
                nc.vector.tensor_scalar_mul(att_d[:], ex[:], rz[:, 0:1])
                # keep-mask: excl[d,t] = sum_{t'} a[d,t'] * (a[d,t'] > a[d,t])
                a_tp = att_d.unsqueeze(1).broadcast_to([DAYS, TOPICS, TOPICS])
                a_t = att_d.unsqueeze(2).broadcast_to([DAYS, TOPICS, TOPICS])
                gtm = pp.tile([DAYS, TOPICS, TOPICS], F32, tag="gtm")
                nc.vector.tensor_tensor(gtm[:], a_tp, a_t, ALU.is_gt)
                nc.vector.tensor_mul(gtm[:], gtm[:], a_tp)
                excl = pp.tile([DAYS, TOPICS], F32, tag="excl")
                nc.vector.tensor_reduce(excl[:], gtm[:], mybir.AxisListType.X, ALU.add)
                keep = pp.tile([DAYS, TOPICS], F32, tag="keep")
                nc.vector.tensor_scalar(keep[:], excl[:], 0.8, scalar2=None,
                                        op0=ALU.is_le)
                wgt = pp.tile([DAYS, TOPICS], BF16, tag="wgt")
                nc.vector.tensor_tensor(wgt[:], keep[:], att_d[:], ALU.mult)
                # back to [1, 600] tp-major: PE transpose + DRAM round trip
                wT_ps = mps.tile([TOPICS, DAYS], BF16, tag="wT")
                nc.tensor.transpose(wT_ps[0:TOPICS, :], wgt[:], ident[0:DAYS, 0:DAYS])
                w20 = pp.tile([TOPICS, DAYS], BF16, tag="w20")
                nc.vector.tensor_copy(w20[:], wT_ps[0:TOPICS, :])
                d600b = dp.tile([B], BF16, tag="d600b")
                nc.sync.dma_start(d600b[:], w20[:])
                wfl = pp.tile([1, B], BF16, tag="wfl")
                nc.sync.dma_start(wfl[:], d600b.rearrange("(x) -> x").unsqueeze(0))
                # broadcast weights to 128 partitions (K=1 ones matmul)
                wb = pp.tile([128, B], BF16, tag="wb")
                for nn in range(2):
                    bb = mps.tile([128, 300], F32, tag="bc")
                    nc.tensor.matmul(bb[:], ones_f[0:1, :],
                                     wfl[0:1, 300 * nn:300 * (nn + 1)],
                                     start=True, stop=True)
                    nc.scalar.activation(wb[:, 300 * nn:300 * (nn + 1)], bb[:], AF.Copy)
                my = pp.tile([128, 2, B], BF16, tag="my")
                nc.vector.tensor_mul(my[:], ytf[:],
                                     wb.unsqueeze(1).broadcast_to([128, 2, B]))
                dh = pp.tile([128, 2, DAYS], F32, tag="dh")
                nc.vector.tensor_reduce(
                    dh[:], my.rearrange("p j (tp d) -> p j d tp", tp=TOPICS),
                    mybir.AxisListType.X, ALU.add)
            ctxC.__exit__(None, None, None)

            # ======== Phase D: fused 2-layer day LSTM + head ========
            ctxD = nc.named_scope("phaseD_day")
            ctxD.__enter__()
            dwih0 = pp.tile([128, 2, 4, DH], F8, tag="dwih0")
            nc.sync.dma_start(dwih0[:],
                              d_wih0_d.ap().rearrange("(j p) g h -> p j g h", p=128))
            dwc = pp.tile([128, 4, 128], F8, tag="dwc")
            nc.sync.dma_start(dwc[:], d_wc_d.ap())
            dseed = pp.tile([128, 4, DAYS], BF16, tag="dseed")
            nc.sync.dma_start(dseed[:], d_seed_d.ap())

            with tc.tile_pool(name="dps", bufs=2, space="PSUM") as dps, \
                 tc.tile_pool(name="dg0", bufs=1, space="PSUM") as dg0p:
                dh_bf = pp.tile([128, 2, DAYS], BF16, tag="dh_bf")
                nc.vector.tensor_copy(dh_bf[:], dh[:])
                g0p = dg0p.tile([DH, 4, DAYS], F32, tag="g0")
                for g in range(4):
                    for j in range(2):
                        nc.tensor.matmul(g0p[0:DH, g, :], dwih0[:, j, g, :],
                                         dh_bf[:, j, :], start=(j == 0),
                                         stop=(j == 1))
                seed = pp.tile([128, 4, DAYS], BF16, tag="seed")
                nc.vector.tensor_copy(seed[:], dseed[:])
                nc.vector.tensor_add(seed[0:DH], seed[0:DH], g0p[0:DH, :, :])

                st = pp.tile([128, 1], BF16, tag="st_day")
                cst = pp.tile([128, 1], BF16, tag="cst_day")
                nc.any.memset(st[:], 0.0)
                nc.any.memset(cst[:], 0.0)
                yd = pp.tile([128, DAYS], F32, tag="yd128")

                for t in range(DAYS + 1):
                    rp = dps.tile([128, 4], F32, tag="rp", name="rp")
                    for g in range(4):
                        nc.tensor.matmul(rp[:, g:g + 1], dwc[:, g, :],
                                         st[:, 0:1], start=True, stop=True,
                                         skip_group_check=True)
                    sl = slice(0, DH) if t == 0 else (
                        slice(DH, 128) if t == DAYS else slice(0, 128))
                    tcol = min(t, DAYS - 1)
                    rp2 = ap_.tile([128, 4], BF16, tag="rp2", name="rp2")
                    nc.vector.tensor_add(rp2[sl], rp[sl], seed[sl, :, tcol])
                    s4 = ap_.tile([128, 4], F32, tag="s4", name="s4")
                    nc.scalar.activation(s4[sl], rp2[sl], AF.Sigmoid, scale=ISC)
                    tgd = ap_.tile([128, 1], BF16, tag="tgd", name="tgd")
                    nc.vector.tensor_scalar(tgd[sl], s4[sl, 2:3], 2.0, -1.0,
                                            op0=ALU.mult, op1=ALU.add)
                    ud = ap_.tile([128, 1], BF16, tag="ud", name="ud")
                    nc.vector.tensor_mul(ud[sl], s4[sl, 0:1], tgd[sl])
                    nc.vector.scalar_tensor_tensor(cst[sl], cst[sl], s4[sl, 1:2],
                                                   ud[sl], op0=ALU.mult,
                                                   op1=ALU.add)
                    tnc = ap_.tile([128, 1], BF16, tag="tnc", name="tnc")
                    nc.scalar.activation(tnc[sl], cst[sl], AF.Tanh)
                    nc.vector.tensor_scalar_mul(st[sl], tnc[sl], s4[sl, 3:4])
                    if t >= 1:
                        nc.vector.tensor_copy(yd[DH:128, t - 1:t], st[DH:128, 0:1])

                hd = st[0:DH, 0:1]           # layer-0 final hidden [64, 1]
                # shift y_day down to partitions 0:64 for the attention tail
                ydl = pp.tile([DH, DAYS], F32, tag="ydl")
                nc.sync.dma_start(ydl[:], yd[DH:128, :])

                # day attention
                w2t = pp.tile([DH, DH], F32, tag="w2t")
                nc.sync.dma_start(w2t[:], w2t_d.ap())
                w2b = pp.tile([DH, 1], F32, tag="w2b")
                nc.sync.dma_start(w2b[:], w2b_d.ap())
                ones64 = pp.tile([1, DH], F32, tag="ones64")
                nc.sync.dma_start(ones64[:], ones_f32_d.ap())

                zp = dps.tile([DH, DAYS], F32, tag="tail_ps")
                nc.tensor.matmul(zp[0:DH, :], w2t[0:DH, :], ydl[0:DH, :],
                                 start=True, stop=True)
                z2 = pp.tile([DH, DAYS], F32, tag="z2")
                nc.scalar.activation(z2[:], zp[0:DH, :], AF.Identity, bias=w2b[:, 0:1])
                p2 = pp.tile([DH, DAYS], F32, tag="p2")
                nc.vector.tensor_mul(p2[:], z2[:], hd.broadcast_to([DH, DAYS]))
                onesp64 = pp.tile([DH, 1], F32, tag="onesp64")
                nc.any.memset(onesp64[:], 1.0)
                s2p = dps.tile([1, DAYS], F32, tag="tail_ps")
                nc.tensor.matmul(s2p[0:1, :], onesp64[0:DH, 0:1], p2[0:DH, :],
                                 start=True, stop=True)
                sc2 = pp.tile([1, DAYS], F32, tag="sc2")
                nc.scalar.activation(sc2[:], s2p[0:1, :], AF.Copy)
                mx2 = pp.tile([1, 1], F32, tag="mx2")
                nc.vector.tensor_reduce(mx2[:], sc2[:], mybir.AxisListType.X, ALU.max)
                nmx2 = pp.tile([1, 1], F32, tag="nmx2")
                nc.scalar.mul(nmx2[:], mx2[:], -1.0)
                e2 = pp.tile([1, DAYS], F32, tag="e2")
                nc.scalar.activation(e2[:], sc2[:], AF.Exp, bias=nmx2[0:1, 0:1])
                z2s = pp.tile([1, 1], F32, tag="z2s")
                nc.vector.tensor_reduce(z2s[:], e2[:], mybir.AxisListType.X, ALU.add)
                rz2 = pp.tile([1, 1], F32, tag="rz2")
                nc.vector.reciprocal(rz2[:], z2s[:])
                at2 = pp.tile([1, DAYS], F32, tag="at2")
                nc.vector.tensor_scalar_mul(at2[:], e2[:], rz2[0:1, 0:1])
                a2p = dps.tile([DH, DAYS], F32, tag="tail_ps")
                nc.tensor.matmul(a2p[0:DH, :], ones64[0:1, :], at2[0:1, :],
                                 start=True, stop=True)
                my2 = pp.tile([DH, DAYS], F32, tag="my2")
                nc.vector.tensor_mul(my2[:], ydl[:], a2p[0:DH, :])
                ctx = pp.tile([DH, 1], F32, tag="ctx")
                nc.vector.tensor_reduce(ctx[:], my2[:], mybir.AxisListType.X, ALU.add)

                # head
                l1t = pp.tile([DH, 48], F32, tag="l1t")
                nc.sync.dma_start(l1t[:], l1t_d.ap())
                l1b = pp.tile([48, 1], F32, tag="l1b")
                nc.sync.dma_start(l1b[:], l1b_d.ap())
                l2t = pp.tile([48, 16], F32, tag="l2t")
                nc.sync.dma_start(l2t[:], l2t_d.ap())
                l2b = pp.tile([16, 1], F32, tag="l2b")
                nc.sync.dma_start(l2b[:], l2b_d.ap())
                hw16 = pp.tile([16, 4], F32, tag="hw16")
                nc.sync.dma_start(hw16[:], hw16_d.ap())
                hw4 = pp.tile([4, 4], F32, tag="hw4")
                nc.sync.dma_start(hw4[:], hw4_d.ap())
                hb = pp.tile([4, 1], F32, tag="hb")
                nc.sync.dma_start(hb[:], hb_d.ap())
                prev = pp.tile([4, 4], F32, tag="prev")
                nc.sync.dma_start(prev[:], prev_d.ap())

                h1p = dps.tile([48, 1], F32, tag="tail_ps")
                nc.tensor.matmul(h1p[0:48, :], l1t[0:DH, :], ctx[0:DH, 0:1],
                                 start=True, stop=True)
                h1 = pp.tile([48, 1], F32, tag="h1")
                nc.scalar.activation(h1[:], h1p[0:48, :], AF.Identity, bias=l1b[:, 0:1])
                h2p = dps.tile([16, 1], F32, tag="tail_ps")
                nc.tensor.matmul(h2p[0:16, :], l2t[0:48, :], h1[0:48, 0:1],
                                 start=True, stop=True)
                h2 = pp.tile([16, 1], F32, tag="h2")
                nc.scalar.activation(h2[:], h2p[0:16, :], AF.Identity, bias=l2b[:, 0:1])
                op_ = dps.tile([4, 1], F32, tag="tail_ps")
                nc.tensor.matmul(op_[0:4, :], hw16[0:16, :], h2[0:16, 0:1],
                                 start=True, stop=True)
                pv = pp.tile([4, 4], F32, tag="pv")
                nc.vector.tensor_mul(pv[:], prev[:], hw4[:])
                pvs = pp.tile([4, 1], F32, tag="pvs")
                nc.vector.tensor_reduce(pvs[:], pv[:], mybir.AxisListType.X, ALU.add)
                r1 = pp.tile([4, 1], F32, tag="r1")
                nc.vector.tensor_add(r1[:], op_[0:4, :], pvs[:])
                res_sb = pp.tile([4, 1], F32, tag="res_sb")
                nc.vector.tensor_add(res_sb[:], r1[:], hb[:])
                nc.sync.dma_start(res_d.ap(), res_sb[:])
            ctxD.__exit__(None, None, None)

    nc.compile()
    return nc


PERM_H = np.arange(4 * H)                              # gate order i,f,g,o kept
PERM_G4 = [0, 1, 2, 3]
# per-gate fp8 scale: 16x, g-gate 32x (tanh(g) = 2*sig(2g)-1 trick)
SC_H = np.repeat([WS, WS, 2 * WS, WS], H)              # [4H] col scale, i,f,g,o
SC_G4 = np.array([WS, WS, 2 * WS, WS])


def _q8(w):
    return np.asarray(w, np.float32).astype(F8H)


def _prep(inputs):
    """Host-side sharding + layout prep (topic-major sequence order)."""
    X = np.asarray(inputs["X"], np.float32)
    xf = np.ascontiguousarray(X.transpose(1, 0, 2, 3)).reshape(B, T, E)
    shared = {}
    wih0 = np.zeros((EP, 4 * H), np.float32)
    wih0[:E] = np.asarray(inputs["txt_Wih0"], np.float32)[PERM_H].T
    wih0[E] = np.asarray(inputs["txt_b0"], np.float32)[PERM_H]
    shared["wih0"] = _q8(wih0 * SC_H)
    shared["whh0"] = _q8(
        np.asarray(inputs["txt_Whh0"], np.float32)[PERM_H].T * SC_H)
    shared["ident"] = np.eye(128, dtype=BF)
    shared["identf"] = np.eye(32, dtype=np.float32)
    shared["ones_p"] = np.ones((128, 1), BF)
    shared["ones_f"] = np.ones((1, 128), BF)
    shared["ones6"] = np.ones((1, B), BF)
    shared["ones_f32"] = np.ones((1, 64), np.float32)
    for nm, w in (("t_wih0", "top_Wih0"), ("t_whh0", "top_Whh0"),
                  ("t_wih1", "top_Wih1"), ("t_whh1", "top_Whh1")):
        shared[nm] = _q8(np.asarray(inputs[w], np.float32)[PERM_H].T * SC_H)
    shared["t_b0r"] = (np.asarray(inputs["top_b0"], np.float32)[PERM_H]
                       * SC_H).reshape(1, 4 * H).astype(BF)
    b1p = (np.asarray(inputs["top_b1"], np.float32)[PERM_H] * SC_H)
    shared["t_b1bc"] = np.ascontiguousarray(np.broadcast_to(
        b1p.reshape(8, 128).T[:, :, None], (128, 8, DAYS))).astype(BF)
    shared["w1t"] = np.asarray(inputs["w1_W"], np.float32).T.astype(BF)
    shared["w1b"] = np.ascontiguousarray(
        np.asarray(inputs["w1_b"], np.float32).reshape(2, 128).T)
    # day LSTM layer 0 input weights [K=256, 4, DH], fp8 x16 (g x32)
    wm = np.asarray(inputs["day_Wih0"], np.float32)
    shared["d_wih0"] = _q8(
        np.ascontiguousarray(wm.reshape(4, DH, H)[PERM_G4].transpose(2, 0, 1))
        * SC_G4[None, :, None])
    # fused-layer combined recurrence weights [k=(h0|h1), 4, m=(L0|L1)]
    whh0d = np.asarray(inputs["day_Whh0"], np.float32).reshape(4, DH, DH)[PERM_G4]
    wih1d = np.asarray(inputs["day_Wih1"], np.float32).reshape(4, DH, DH)[PERM_G4]
    whh1d = np.asarray(inputs["day_Whh1"], np.float32).reshape(4, DH, DH)[PERM_G4]
    wc = np.zeros((128, 4, 128), np.float32)
    wc[0:DH, :, 0:DH] = whh0d.transpose(2, 0, 1)
    wc[0:DH, :, DH:128] = wih1d.transpose(2, 0, 1)
    wc[DH:128, :, DH:128] = whh1d.transpose(2, 0, 1)
    shared["d_wc"] = _q8(wc * SC_G4[None, :, None])
    b0d = np.asarray(inputs["day_b0"], np.float32).reshape(4, DH)[PERM_G4]
    b1d = np.asarray(inputs["day_b1"], np.float32).reshape(4, DH)[PERM_G4]
    seed = np.zeros((128, 4, DAYS), np.float32)
    seed[0:DH] = (b0d.T * SC_G4[None, :])[:, :, None]
    seed[DH:128] = (b1d.T * SC_G4[None, :])[:, :, None]
    shared["d_seed"] = seed.astype(BF)
    shared["w2t"] = np.ascontiguousarray(np.asarray(inputs["w2_W"], np.float32).T)
    shared["w2b"] = np.asarray(inputs["w2_b"], np.float32).reshape(DH, 1)
    shared["l1t"] = np.ascontiguousarray(np.asarray(inputs["lin1_W"], np.float32).T)
    shared["l1b"] = np.asarray(inputs["lin1_b"], np.float32).reshape(48, 1)
    shared["l2t"] = np.ascontiguousarray(np.asarray(inputs["lin2_W"], np.float32).T)
    shared["l2b"] = np.asarray(inputs["lin2_b"], np.float32).reshape(16, 1)
    hw = np.asarray(inputs["head_W"], np.float32)
    shared["hw16"] = np.ascontiguousarray(hw[:, :16].T)
    shared["hw4"] = np.ascontiguousarray(hw[:, 16:])
    shared["hb"] = np.asarray(inputs["head_b"], np.float32).reshape(4, 1)
    shared["prev"] = np.asarray(inputs["previous_labels"], np.float32)

    in_maps = []
    for r in range(NC_):
        xr = xf[BC * r:BC * (r + 1)]                    # [75, 128, 300]
        xe = np.zeros((T, EP, BCP), np.float32)
        xe[:, :E, 0:BC] = xr.transpose(1, 2, 0)
        xe[:, E, 0:BC] = 1.0
        # super-chunk layout matching on-chip tiles: [sc, p, t, k, b]
        xp = np.ascontiguousarray(
            xe.reshape(T // 8, 8, 3, 128, BCP)
              .transpose(0, 3, 1, 2, 4)).astype(BF)
        m = dict(shared)
        m["x"] = xp
        in_maps.append(m)
    return in_maps


def kernel(**inputs) -> np.ndarray:
    if "nc" not in _cache:
        _cache["nc"] = build()
    nc = _cache["nc"]
    in_maps = _prep(inputs)
    import os
    trace = bool(os.environ.get("KERNEL_TRACE"))
    res = run_bass_kernel_spmd(nc, in_maps, core_ids=list(range(NC_)),
                               trace=trace)
    _cache["last_results"] = res
    return np.asarray(res.results[0]["res"], np.float32)


# revision 15
# speedup vs baseline: 1.2538x; 1.2538x over previous
"""Trainium2 Bass kernel for nn_Model_26439818674684.

Architecture (from the reference):
  - text LSTM over 600=30*20 sequences of len 128 (E=300 -> H=256). Only
    LAYER 0's final hidden state is consumed downstream, so only layer 0 is
    computed.
  - topic LSTM (2 layers, batch=30 days, T=20 topics, H=256)
  - per-day attention with a sorted-cumsum keep mask (sort-free via pairwise
    comparisons)
  - day LSTM (2 layers, batch=1, T=30, 256 -> 64), small attention + head.

Key structure vs the v0 kernel:
  - Sequences are sharded TOPIC-MAJOR (600 cols ordered tp*30+d) so the topic
    LSTM's per-step batch is a contiguous 30-column slice.
  - Text LSTM: 75 seqs/core padded to 76 = 2 groups of 38, software-pipelined
    so one group's matmuls overlap the other group's activation chain.
  - Input gates accumulate directly in PSUM; Whh recurrence matmuls accumulate
    on top (start=False). No identity-seed matmuls, no PSUM->SBUF gate copies.
  - Single-sigmoid trick: g-gate rows pre-scaled by 2, tanh(g) = 2*sig(2g)-1.
    One Sigmoid activation covers all 4 gates.
  - LSTM weights in fp8e4m3 scaled by 16 (sigmoid activation applies 1/16),
    activations bf16. Validated host-side: final rel err ~6e-5 (tol 2e-2).
  - Day LSTM: both layers fused into one 128-partition instruction stream
    (h0 on partitions 0:64, h1 on 64:128), halving the serial round count.
"""
import sys
sys.path.insert(0, '/opt/trn_rl_repo')

import numpy as np
import ml_dtypes

import concourse.bass as bass
import concourse.tile as tile
from concourse import bacc, mybir
from concourse.bass_utils import run_bass_kernel_spmd

F32 = mybir.dt.float32
BF16 = mybir.dt.bfloat16
F8 = mybir.dt.float8e4
AF = mybir.ActivationFunctionType
ALU = mybir.AluOpType
BF = ml_dtypes.bfloat16
F8H = ml_dtypes.float8_e4m3fn

NC_ = 8
DAYS, TOPICS, T, E, H, DH = 30, 20, 128, 300, 256, 64
B = DAYS * TOPICS          # 600
BC = B // NC_              # 75 sequences per core
GB = 38                    # group batch (75 padded to 76 = 2*38)
BCP = 2 * GB
EP = 384                   # E padded to 3 K-tiles (row 300 = bias ones-row)
WS = 16.0                  # fp8 weight scale (sigmoid applies 1/WS)
ISC = 1.0 / WS
CH = 4                     # topic L0 input-gate chunk (steps)

_cache = {}


def build():
    nc = bacc.Bacc("TRN2", target_bir_lowering=False, debug=False,
                   enable_asserts=False, num_devices=NC_)

    # ---------------- DRAM I/O ----------------
    x_d = nc.dram_tensor("x", [T // 8, 128, 8, 3, BCP], BF16,
                         kind="ExternalInput")
    wih0_d = nc.dram_tensor("wih0", [EP, 4 * H], F8, kind="ExternalInput")
    whh0_d = nc.dram_tensor("whh0", [H, 4 * H], F8, kind="ExternalInput")
    ident_d = nc.dram_tensor("ident", [128, 128], BF16, kind="ExternalInput")
    identf_d = nc.dram_tensor("identf", [32, 32], F32, kind="ExternalInput")
    ones_p_d = nc.dram_tensor("ones_p", [128, 1], BF16, kind="ExternalInput")
    ones_f_d = nc.dram_tensor("ones_f", [1, 128], BF16, kind="ExternalInput")
    ones6_d = nc.dram_tensor("ones6", [1, B], BF16, kind="ExternalInput")
    ones_f32_d = nc.dram_tensor("ones_f32", [1, 64], F32, kind="ExternalInput")
    t_wih0_d = nc.dram_tensor("t_wih0", [H, 4 * H], F8, kind="ExternalInput")
    t_whh0_d = nc.dram_tensor("t_whh0", [H, 4 * H], F8, kind="ExternalInput")
    t_wih1_d = nc.dram_tensor("t_wih1", [H, 4 * H], F8, kind="ExternalInput")
    t_whh1_d = nc.dram_tensor("t_whh1", [H, 4 * H], F8, kind="ExternalInput")
    t_b0r_d = nc.dram_tensor("t_b0r", [1, 4 * H], BF16, kind="ExternalInput")
    t_b1bc_d = nc.dram_tensor("t_b1bc", [128, 8, DAYS], BF16,
                              kind="ExternalInput")
    w1t_d = nc.dram_tensor("w1t", [H, H], BF16, kind="ExternalInput")
    w1b_d = nc.dram_tensor("w1b", [128, 2], F32, kind="ExternalInput")
    d_wih0_d = nc.dram_tensor("d_wih0", [H, 4, DH], F8, kind="ExternalInput")
    d_wc_d = nc.dram_tensor("d_wc", [128, 4, 128], F8, kind="ExternalInput")
    d_seed_d = nc.dram_tensor("d_seed", [128, 4, DAYS], BF16,
                              kind="ExternalInput")
    w2t_d = nc.dram_tensor("w2t", [DH, DH], F32, kind="ExternalInput")
    w2b_d = nc.dram_tensor("w2b", [DH, 1], F32, kind="ExternalInput")
    l1t_d = nc.dram_tensor("l1t", [DH, 48], F32, kind="ExternalInput")
    l1b_d = nc.dram_tensor("l1b", [48, 1], F32, kind="ExternalInput")
    l2t_d = nc.dram_tensor("l2t", [48, 16], F32, kind="ExternalInput")
    l2b_d = nc.dram_tensor("l2b", [16, 1], F32, kind="ExternalInput")
    hw16_d = nc.dram_tensor("hw16", [16, 4], F32, kind="ExternalInput")
    hw4_d = nc.dram_tensor("hw4", [4, 4], F32, kind="ExternalInput")
    hb_d = nc.dram_tensor("hb", [4, 1], F32, kind="ExternalInput")
    prev_d = nc.dram_tensor("prev", [4, 4], F32, kind="ExternalInput")
    res_d = nc.dram_tensor("res", [4, 1], F32, kind="ExternalOutput")

    with tile.TileContext(nc) as tc:
        with tc.tile_pool(name="persist", bufs=1) as pp, \
             tc.tile_pool(name="act", bufs=4) as ap_, \
             tc.tile_pool(name="dram", bufs=1, space="DRAM") as dp:

            # ======== Phase A: text LSTM layer 0, 2x38 pipelined groups ====
            wih = pp.tile([128, 3, 4 * H], F8, tag="wih")
            nc.sync.dma_start(wih[:], wih0_d.ap().rearrange("(k p) m -> p k m", p=128))
            whh = pp.tile([128, 2, 4 * H], F8, tag="whh")
            nc.sync.dma_start(whh[:], whh0_d.ap().rearrange("(j p) m -> p j m", p=128))

            hb_t = pp.tile([128, 2, BCP], BF16, tag="hb_t")
            cb_t = pp.tile([128, 2, BCP], BF16, tag="cb_t")
            nc.any.memset(cb_t[:], 0.0)

            ctxA = nc.named_scope("phaseA_text")
            ctxA.__enter__()
            with tc.tile_pool(name="xin", bufs=3) as xip, \
                 tc.tile_pool(name="wrm", bufs=1, space="PSUM") as wrm, \
                 tc.tile_pool(name="gps", bufs=3, space="PSUM") as gps:
                xts = {}

                def rec_mms(gp, m_lo, m_hi):
                    for m in range(m_lo, m_hi):
                        for j in range(2):
                            nc.tensor.matmul(
                                gp[:, m // 4, m % 4, 0:BCP],
                                whh[:, j, 128 * m:128 * (m + 1)],
                                hb_t[:, j, :], start=False, stop=(j == 1),
                                skip_group_check=True)

                for t in range(T):
                    if t % 8 == 0:
                        xt = xip.tile([128, 8, 3, BCP], BF16, tag="xt")
                        nc.sync.dma_start(xt[:], x_d.ap()[t // 8])
                        xts[t // 8] = xt
                    xt = xts[t // 8]
                    gp = gps.tile([128, 2, 4, 128], F32, tag="g", name="gp")
                    for m in range(8):
                        dst = gp[:, m // 4, m % 4, 0:BCP]
                        for k in range(3):
                            nc.tensor.matmul(
                                dst, wih[:, k, 128 * m:128 * (m + 1)],
                                xt[:, t % 8, k, :], start=(k == 0),
                                stop=(k == 2 and t == 0),
                                skip_group_check=True)
                    # recurrence accumulates on top of the input gates.
                    # m-tiles 0:6 = i,f,g gates; 6:8 = o gates. The i/f/g
                    # sigmoid fires as soon as the first 12 recurrence MMs
                    # land; the o-gate MMs + sigmoid overlap the vector chain.
                    if t > 0:
                        rec_mms(gp, 0, 6)
                    gpm = gp.rearrange("p a m x -> p (a m) x")
                    s6 = ap_.tile([128, 6, BCP], BF16, tag="s6", name="s6")
                    nc.scalar.activation(s6[:], gpm[:, 0:6, 0:BCP], AF.Sigmoid,
                                         scale=ISC)
                    if t > 0:
                        rec_mms(gp, 6, 8)
                    s2 = ap_.tile([128, 2, BCP], BF16, tag="s2", name="s2")
                    nc.scalar.activation(s2[:], gpm[:, 6:8, 0:BCP], AF.Sigmoid,
                                         scale=ISC)
                    tg = ap_.tile([128, 2, BCP], BF16, tag="tg", name="tg")
                    nc.vector.tensor_scalar(tg[:], s6[:, 4:6], 2.0, -1.0,
                                            op0=ALU.mult, op1=ALU.add)
                    u = ap_.tile([128, 2, BCP], BF16, tag="u", name="u")
                    nc.vector.tensor_mul(u[:], s6[:, 0:2], tg[:])
                    v = ap_.tile([128, 2, BCP], BF16, tag="v", name="v")
                    nc.vector.tensor_mul(v[:], s6[:, 2:4], cb_t[:])
                    nc.vector.tensor_add(cb_t[:], u[:], v[:])
                    tc_ = ap_.tile([128, 2, BCP], BF16, tag="tc", name="tc")
                    nc.scalar.activation(tc_[:], cb_t[:], AF.Tanh)
                    nc.vector.tensor_mul(hb_t[:], s2[:], tc_[:])
                    # warm-keepers: independent matmuls that run while the PE
                    # would otherwise idle waiting for h, keeping the HAM
                    # clock gate at full rate.
                    scr = wrm.tile([128, 512], F32, tag="scr", name="scr")
                    for dmy in range(18):
                        nc.tensor.matmul(scr[:, 0:BCP],
                                         whh[:, dmy % 2, 0:128],
                                         xt[:, t % 8, dmy % 3, :],
                                         start=True, stop=True,
                                         skip_group_check=True)

            ctxA.__exit__(None, None, None)
            # ======== Phase B: AllGather (topic-major columns) ========
            ctxB = nc.named_scope("phaseB_gather")
            ctxB.__enter__()
            hl = dp.tile([2, 128, BC], BF16, tag="hl")
            nc.sync.dma_start(hl[:],
                              hb_t[:, :, 0:BC].rearrange("p j b -> j p b"))
            gat = dp.tile([NC_, 2, 128, BC], BF16, tag="gat")
            nc.gpsimd.collective_compute(
                "AllGather", ALU.bypass,
                replica_groups=[list(range(NC_))],
                ins=[hl.opt()], outs=[gat.opt()])
            h_all = pp.tile([128, 2, B], BF16, tag="h_all")
            for r_ in range(NC_):
                nc.sync.dma_start(h_all[:, :, BC * r_:BC * (r_ + 1)],
                                  gat[r_].rearrange("j p b -> p j b"))
            ctxB.__exit__(None, None, None)

            # ======== Phase B2: topic LSTM (2 layers, 20 steps, N=30) ======
            ctxT = nc.named_scope("phaseB_topic")
            ctxT.__enter__()
            ones6 = pp.tile([1, B], BF16, tag="ones6")
            nc.sync.dma_start(ones6[:], ones6_d.ap())
            ones_f = pp.tile([1, 128], BF16, tag="ones_f")
            nc.sync.dma_start(ones_f[:], ones_f_d.ap())
            tw = {}
            for nm, d in (("t_wih0", t_wih0_d), ("t_whh0", t_whh0_d),
                          ("t_wih1", t_wih1_d), ("t_whh1", t_whh1_d)):
                tw[nm] = pp.tile([128, 2, 4 * H], F8, tag=nm, name=nm)
                nc.sync.dma_start(tw[nm][:],
                                  d.ap().rearrange("(j p) m -> p j m", p=128))
            b0r = pp.tile([1, 4 * H], BF16, tag="b0r")
            nc.sync.dma_start(b0r[:], t_b0r_d.ap())
            b1bc = pp.tile([128, 8, DAYS], BF16, tag="b1bc")
            nc.sync.dma_start(b1bc[:], t_b1bc_d.ap())

            y0 = pp.tile([128, 2, TOPICS, DAYS], BF16, tag="y0")
            ytop = pp.tile([128, 2, TOPICS, DAYS], BF16, tag="ytop")
            z30 = pp.tile([128, 2, DAYS], BF16, tag="z30")
            ct0 = pp.tile([128, 2, DAYS], BF16, tag="ct0")
            ct1 = pp.tile([128, 2, DAYS], BF16, tag="ct1")
            for ap0 in (z30, ct0, ct1):
                nc.any.memset(ap0[:], 0.0)

            with tc.tile_pool(name="tl0", bufs=2, space="PSUM") as tl0, \
                 tc.tile_pool(name="tl1", bufs=2, space="PSUM") as tl1:
                l0ch = {}

                def l0_chunk(c):
                    # input gates for CH steps: 120 contiguous tp-major cols
                    pc = tl0.tile([128, 2, 4, 128], F32, tag="pc",
                                  name="pc")
                    cs = slice(CH * DAYS * c, CH * DAYS * (c + 1))
                    for m in range(8):
                        dst = pc[:, m // 4, m % 4, 0:CH * DAYS]
                        for j in range(2):
                            nc.tensor.matmul(
                                dst, tw["t_wih0"][:, j, 128 * m:128 * (m + 1)],
                                h_all[:, j, cs], start=(j == 0), stop=False,
                                skip_group_check=True)
                        nc.tensor.matmul(
                            dst, b0r[0:1, 128 * m:128 * (m + 1)],
                            ones6[0:1, 0:CH * DAYS], start=False, stop=False,
                            skip_group_check=True)
                    l0ch[c] = pc

                def t_sig(gin, lyr):
                    s8 = ap_.tile([128, 8, DAYS], BF16, tag=f"ts8{lyr}",
                                  name="s8")
                    nc.scalar.activation(s8[:], gin, AF.Sigmoid, scale=ISC)
                    return s8

                def t_cell(s8, c_st, lyr):
                    tg = ap_.tile([128, 2, DAYS], BF16, tag=f"ttg{lyr}",
                                  name="tg")
                    nc.vector.tensor_scalar(tg[:], s8[:, 4:6], 2.0, -1.0,
                                            op0=ALU.mult, op1=ALU.add)
                    u = ap_.tile([128, 2, DAYS], BF16, tag=f"tu{lyr}", name="u")
                    nc.vector.tensor_mul(u[:], s8[:, 0:2], tg[:])
                    v = ap_.tile([128, 2, DAYS], BF16, tag=f"tv{lyr}", name="v")
                    nc.vector.tensor_mul(v[:], s8[:, 2:4], c_st[:])
                    nc.vector.tensor_add(c_st[:], u[:], v[:])

                def t_tanh(c_st, lyr):
                    tc_ = ap_.tile([128, 2, DAYS], BF16, tag=f"ttc{lyr}",
                                   name="tc")
                    nc.scalar.activation(tc_[:], c_st[:], AF.Tanh)
                    return tc_

                def l0_mms(t):
                    pc = l0ch[t // CH]
                    tl = t % CH
                    gs = slice(DAYS * tl, DAYS * (tl + 1))
                    hprev = z30 if t == 0 else y0[:, :, t - 1, :]
                    for m in range(8):
                        for j in range(2):
                            nc.tensor.matmul(
                                pc[:, m // 4, m % 4, gs],
                                tw["t_whh0"][:, j, 128 * m:128 * (m + 1)],
                                hprev[:, j, :], start=False, stop=(j == 1),
                                skip_group_check=True)
                    return pc.rearrange("p a m x -> p (a m) x")[:, :, gs]

                def l1_mms(t):
                    p1 = tl1.tile([128, 2, 4, 64], F32, tag="p1", name="p1")
                    hprev = z30 if t == 0 else ytop[:, :, t - 1, :]
                    for m in range(8):
                        dst = p1[:, m // 4, m % 4, 0:DAYS]
                        for j in range(2):
                            nc.tensor.matmul(
                                dst, tw["t_wih1"][:, j, 128 * m:128 * (m + 1)],
                                y0[:, j, t, :], start=(j == 0), stop=False,
                                skip_group_check=True)
                        for j in range(2):
                            nc.tensor.matmul(
                                dst, tw["t_whh1"][:, j, 128 * m:128 * (m + 1)],
                                hprev[:, j, :], start=False, stop=(j == 1),
                                skip_group_check=True)
                    return p1

                def l1_bias(p1):
                    g1 = ap_.tile([128, 8, DAYS], BF16, tag="g1", name="g1")
                    nc.vector.tensor_add(
                        g1[:], p1.rearrange("p a m x -> p (a m) x")[:, :, 0:DAYS],
                        b1bc[:])
                    return g1

                l0_chunk(0)
                l0_chunk(1)
                # steady state: L0 step t and L1 step t-1 pipelined with
                # engine streams interleaved (sig0, sig1, cell0, cell1, ...)
                g0in = l0_mms(0)
                s0 = t_sig(g0in, 0)
                t_cell(s0, ct0, 0)
                tc0 = t_tanh(ct0, 0)
                nc.vector.tensor_mul(y0[:, :, 0, :], s0[:, 6:8], tc0[:])
                for t in range(1, TOPICS):
                    if t % CH == 0 and t // CH + 1 < TOPICS // CH:
                        l0_chunk(t // CH + 1)
                    # L0(t) chain runs to completion ASAP (it gates the next
                    # step); L1(t-1) trails, filling the engines.
                    g0in = l0_mms(t)
                    s0 = t_sig(g0in, 0)
                    p1 = l1_mms(t - 1)
                    t_cell(s0, ct0, 0)
                    tc0 = t_tanh(ct0, 0)
                    nc.vector.tensor_mul(y0[:, :, t, :], s0[:, 6:8], tc0[:])
                    s1 = t_sig(l1_bias(p1), 1)
                    t_cell(s1, ct1, 1)
                    tc1 = t_tanh(ct1, 1)
                    nc.vector.tensor_mul(ytop[:, :, t - 1, :], s1[:, 6:8],
                                         tc1[:])
                p1 = l1_mms(TOPICS - 1)
                s1 = t_sig(l1_bias(p1), 1)
                t_cell(s1, ct1, 1)
                tc1 = t_tanh(ct1, 1)
                nc.vector.tensor_mul(ytop[:, :, TOPICS - 1, :], s1[:, 6:8],
                                     tc1[:])
            ctxT.__exit__(None, None, None)

            # ======== Phase C: topic attention (tp-major cols) ========
            ctxC = nc.named_scope("phaseC_attn")
            ctxC.__enter__()
            w1t = pp.tile([128, 2, H], BF16, tag="w1t")
            nc.sync.dma_start(w1t[:], w1t_d.ap().rearrange("(j p) m -> p j m", p=128))
            w1b = pp.tile([128, 2], F32, tag="w1b")
            nc.sync.dma_start(w1b[:], w1b_d.ap())
            ones_p = pp.tile([128, 1], BF16, tag="ones_p")
            nc.sync.dma_start(ones_p[:], ones_p_d.ap())
            ident = pp.tile([128, 128], BF16, tag="ident")
            nc.sync.dma_start(ident[:], ident_d.ap())
            identf = pp.tile([32, 32], F32, tag="identf")
            nc.sync.dma_start(identf[:], identf_d.ap())

            h_top = y0[:, :, TOPICS - 1, :]
            ytf = ytop.rearrange("p j t d -> p j (t d)")
            with tc.tile_pool(name="cps", bufs=2, space="PSUM") as cps, \
                 tc.tile_pool(name="mps", bufs=1, space="PSUM") as mps, \
                 tc.tile_pool(name="scps", bufs=1, space="PSUM") as scps:
                z = pp.tile([128, 2, B], F32, tag="z")
                for mi in range(2):
                    for nn in range(2):
                        cs = slice(300 * nn, 300 * (nn + 1))
                        pt = cps.tile([128, 300], F32, tag="zps")
                        for j in range(2):
                            nc.tensor.matmul(pt[:], w1t[:, j, 128 * mi:128 * (mi + 1)],
                                             ytf[:, j, cs], start=(j == 0), stop=(j == 1))
                        nc.scalar.activation(z[:, mi, cs], pt[:], AF.Identity,
                                             bias=w1b[:, mi:mi + 1])
                prod = pp.tile([128, 2, TOPICS, DAYS], BF16, tag="prod")
                z_r = z.rearrange("p j (tp d) -> p j tp d", tp=TOPICS)
                nc.vector.tensor_mul(
                    prod[:], z_r[:],
                    h_top.unsqueeze(2).broadcast_to([128, 2, TOPICS, DAYS]))
                prodf = prod.rearrange("p j tp d -> p j (tp d)")
                sc_ps = scps.tile([1, 2, 512], F32, tag="sc")
                for nn in range(2):
                    for j in range(2):
                        nc.tensor.matmul(sc_ps[0:1, nn, 0:300], ones_p[:, 0:1],
                                         prodf[:, j, 300 * nn:300 * (nn + 1)],
                                         start=(j == 0), stop=(j == 1))
                sc = pp.tile([1, B], F32, tag="sc_sb")
                nc.scalar.activation(sc.rearrange("p (nn x) -> p nn x", nn=2),
                                     sc_ps[0:1, :, 0:300], AF.Copy)
                # -> [30 days partitions, 20 topics] via DRAM + PE transpose
                d600 = dp.tile([B], F32, tag="d600")
                nc.sync.dma_start(d600[:], sc[0:1, :])
                sc20 = pp.tile([TOPICS, DAYS], F32, tag="sc20")
                nc.sync.dma_start(sc20[:], d600.rearrange("(tp d) -> tp d", tp=TOPICS))
                scT_ps = mps.tile([DAYS, TOPICS], F32, tag="scT")
                nc.tensor.transpose(scT_ps[0:DAYS, :], sc20[:], identf[0:TOPICS, 0:TOPICS])
                scT = pp.tile([DAYS, TOPICS], F32, tag="scT_sb")
                nc.vector.tensor_copy(scT[:], scT_ps[0:DAYS, :])
                # per-day softmax over topics (free dim)
                mx = pp.tile([DAYS, 1], F32, tag="mx")
                nc.vector.tensor_reduce(mx[:], scT[:], mybir.AxisListType.X, ALU.max)
                nmx = pp.tile([DAYS, 1], F32, tag="nmx")
                nc.scalar.mul(nmx[:], mx[:], -1.0)
                ex = pp.tile([DAYS, TOPICS], F32, tag="ex")
                nc.scalar.activation(ex[:], scT[:], AF.Exp, bias=nmx[:, 0:1])
                zs = pp.tile([DAYS, 1], F32, tag="zs")
                nc.vector.tensor_reduce(zs[:], ex[:], mybir.AxisListType.X, ALU.add)
                rz = pp.tile([DAYS, 1], F32, tag="rz")
                nc.vector.reciprocal(rz[:], zs[:])
                att_d = pp.tile([DAYS, TOPICS], F32, tag="att_d")
                nc.vector.tensor_scalar_mul(att_d[:], ex[:], rz[:, 0:1])
                # keep-mask: excl[d,t] = sum_{t'} a[d,t'] * (a[d,t'] > a[d,t])
                a_tp = att_d.unsqueeze(1).broadcast_to([DAYS, TOPICS, TOPICS])
                a_t = att_d.unsqueeze(2).broadcast_to([DAYS, TOPICS, TOPICS])
                gtm = pp.tile([DAYS, TOPICS, TOPICS], F32, tag="gtm")
                nc.vector.tensor_tensor(gtm[:], a_tp, a_t, ALU.is_gt)
                nc.vector.tensor_mul(gtm[:], gtm[:], a_tp)
                excl = pp.tile([DAYS, TOPICS], F32, tag="excl")
                nc.vector.tensor_reduce(excl[:], gtm[:], mybir.AxisListType.X, ALU.add)
                keep = pp.tile([DAYS, TOPICS], F32, tag="keep")
                nc.vector.tensor_scalar(keep[:], excl[:], 0.8, scalar2=None,
                                        op0=ALU.is_le)
                wgt = pp.tile([DAYS, TOPICS], BF16, tag="wgt")
                nc.vector.tensor_tensor(wgt[:], keep[:], att_d[:], ALU.mult)
                # back to [1, 600] tp-major: PE transpose + DRAM round trip
                wT_ps = mps.tile([TOPICS, DAYS], BF16, tag="wT")
                nc.tensor.transpose(wT_ps[0:TOPICS, :], wgt[:], ident[0:DAYS, 0:DAYS])
                w20 = pp.tile([TOPICS, DAYS], BF16, tag="w20")
                nc.vector.tensor_copy(w20[:], wT_ps[0:TOPICS, :])
                d600b = dp.tile([B], BF16, tag="d600b")
                nc.sync.dma_start(d600b[:], w20[:])
                wfl = pp.tile([1, B], BF16, tag="wfl")
                nc.sync.dma_start(wfl[:], d600b.rearrange("(x) -> x").unsqueeze(0))
                # broadcast weights to 128 partitions (K=1 ones matmul)
                wb = pp.tile([128, B], BF16, tag="wb")
                for nn in range(2):
                    bb = mps.tile([128, 300], F32, tag="bc")
                    nc.tensor.matmul(bb[:], ones_f[0:1, :],
                                     wfl[0:1, 300 * nn:300 * (nn + 1)],
                                     start=True, stop=True)
                    nc.scalar.activation(wb[:, 300 * nn:300 * (nn + 1)], bb[:], AF.Copy)
                my = pp.tile([128, 2, B], BF16, tag="my")
                nc.vector.tensor_mul(my[:], ytf[:],
                                     wb.unsqueeze(1).broadcast_to([128, 2, B]))
                dh = pp.tile([128, 2, DAYS], F32, tag="dh")
                nc.vector.tensor_reduce(
                    dh[:], my.rearrange("p j (tp d) -> p j d tp", tp=TOPICS),
                    mybir.AxisListType.X, ALU.add)
            ctxC.__exit__(None, None, None)

            # ======== Phase D: fused 2-layer day LSTM + head ========
            ctxD = nc.named_scope("phaseD_day")
            ctxD.__enter__()
            dwih0 = pp.tile([128, 2, 4, DH], F8, tag="dwih0")
            nc.sync.dma_start(dwih0[:],
                              d_wih0_d.ap().rearrange("(j p) g h -> p j g h", p=128))
            dwc = pp.tile([128, 4, 128], F8, tag="dwc")
            nc.sync.dma_start(dwc[:], d_wc_d.ap())
            dseed = pp.tile([128, 4, DAYS], BF16, tag="dseed")
            nc.sync.dma_start(dseed[:], d_seed_d.ap())

            with tc.tile_pool(name="dps", bufs=2, space="PSUM") as dps, \
                 tc.tile_pool(name="dg0", bufs=1, space="PSUM") as dg0p:
                dh_bf = pp.tile([128, 2, DAYS], BF16, tag="dh_bf")
                nc.vector.tensor_copy(dh_bf[:], dh[:])
                g0p = dg0p.tile([DH, 4, DAYS], F32, tag="g0")
                for g in range(4):
                    for j in range(2):
                        nc.tensor.matmul(g0p[0:DH, g, :], dwih0[:, j, g, :],
                                         dh_bf[:, j, :], start=(j == 0),
                                         stop=(j == 1))
                seed = pp.tile([128, 4, DAYS], BF16, tag="seed")
                nc.vector.tensor_copy(seed[:], dseed[:])
                nc.vector.tensor_add(seed[0:DH], seed[0:DH], g0p[0:DH, :, :])

                st = pp.tile([128, 1], BF16, tag="st_day")
                cst = pp.tile([128, 1], BF16, tag="cst_day")
                nc.any.memset(st[:], 0.0)
                nc.any.memset(cst[:], 0.0)
                yd = pp.tile([128, DAYS], F32, tag="yd128")

                for t in range(DAYS + 1):
                    rp = dps.tile([128, 4], F32, tag="rp", name="rp")
                    for g in range(4):
                        nc.tensor.matmul(rp[:, g:g + 1], dwc[:, g, :],
                                         st[:, 0:1], start=True, stop=True,
                                         skip_group_check=True)
                    sl = slice(0, DH) if t == 0 else (
                        slice(DH, 128) if t == DAYS else slice(0, 128))
                    tcol = min(t, DAYS - 1)
                    rp2 = ap_.tile([128, 4], BF16, tag="rp2", name="rp2")
                    nc.vector.tensor_add(rp2[sl], rp[sl], seed[sl, :, tcol])
                    s4 = ap_.tile([128, 4], F32, tag="s4", name="s4")
                    nc.scalar.activation(s4[sl], rp2[sl], AF.Sigmoid, scale=ISC)
                    tgd = ap_.tile([128, 1], BF16, tag="tgd", name="tgd")
                    nc.vector.tensor_scalar(tgd[sl], s4[sl, 2:3], 2.0, -1.0,
                                            op0=ALU.mult, op1=ALU.add)
                    ud = ap_.tile([128, 1], BF16, tag="ud", name="ud")
                    nc.vector.tensor_mul(ud[sl], s4[sl, 0:1], tgd[sl])
                    nc.vector.scalar_tensor_tensor(cst[sl], cst[sl], s4[sl, 1:2],
                                                   ud[sl], op0=ALU.mult,
                                                   op1=ALU.add)
                    tnc = ap_.tile([128, 1], BF16, tag="tnc", name="tnc")
                    nc.scalar.activation(tnc[sl], cst[sl], AF.Tanh)
                    nc.vector.tensor_scalar_mul(st[sl], tnc[sl], s4[sl, 3:4])
                    if t >= 1:
                        nc.vector.tensor_copy(yd[DH:128, t - 1:t], st[DH:128, 0:1])

                hd = st[0:DH, 0:1]           # layer-0 final hidden [64, 1]
                # shift y_day down to partitions 0:64 for the attention tail
                ydl = pp.tile([DH, DAYS], F32, tag="ydl")
                nc.sync.dma_start(ydl[:], yd[DH:128, :])

                # day attention
                w2t = pp.tile([DH, DH], F32, tag="w2t")
                nc.sync.dma_start(w2t[:], w2t_d.ap())
                w2b = pp.tile([DH, 1], F32, tag="w2b")
                nc.sync.dma_start(w2b[:], w2b_d.ap())
                ones64 = pp.tile([1, DH], F32, tag="ones64")
                nc.sync.dma_start(ones64[:], ones_f32_d.ap())

                zp = dps.tile([DH, DAYS], F32, tag="tail_ps")
                nc.tensor.matmul(zp[0:DH, :], w2t[0:DH, :], ydl[0:DH, :],
                                 start=True, stop=True)
                z2 = pp.tile([DH, DAYS], F32, tag="z2")
                nc.scalar.activation(z2[:], zp[0:DH, :], AF.Identity, bias=w2b[:, 0:1])
                p2 = pp.tile([DH, DAYS], F32, tag="p2")
                nc.vector.tensor_mul(p2[:], z2[:], hd.broadcast_to([DH, DAYS]))
                onesp64 = pp.tile([DH, 1], F32, tag="onesp64")
                nc.any.memset(onesp64[:], 1.0)
                s2p = dps.tile([1, DAYS], F32, tag="tail_ps")
                nc.tensor.matmul(s2p[0:1, :], onesp64[0:DH, 0:1], p2[0:DH, :],
                                 start=True, stop=True)
                sc2 = pp.tile([1, DAYS], F32, tag="sc2")
                nc.scalar.activation(sc2[:], s2p[0:1, :], AF.Copy)
                mx2 = pp.tile([1, 1], F32, tag="mx2")
                nc.vector.tensor_reduce(mx2[:], sc2[:], mybir.AxisListType.X, ALU.max)
                nmx2 = pp.tile([1, 1], F32, tag="nmx2")
                nc.scalar.mul(nmx2[:], mx2[:], -1.0)
                e2 = pp.tile([1, DAYS], F32, tag="e2")
                nc.scalar.activation(e2[:], sc2[:], AF.Exp, bias=nmx2[0:1, 0:1])
                z2s = pp.tile([1, 1], F32, tag="z2s")
                nc.vector.tensor_reduce(z2s[:], e2[:], mybir.AxisListType.X, ALU.add)
                rz2 = pp.tile([1, 1], F32, tag="rz2")
                nc.vector.reciprocal(rz2[:], z2s[:])
                at2 = pp.tile([1, DAYS], F32, tag="at2")
                nc.vector.tensor_scalar_mul(at2[:], e2[:], rz2[0:1, 0:1])
                a2p = dps.tile([DH, DAYS], F32, tag="tail_ps")
                nc.tensor.matmul(a2p[0:DH, :], ones64[0:1, :], at2[0:1, :],
                                 start=True, stop=True)
                my2 = pp.tile([DH, DAYS], F32, tag="my2")
                nc.vector.tensor_mul(my2[:], ydl[:], a2p[0:DH, :])
                ctx = pp.tile([DH, 1], F32, tag="ctx")
                nc.vector.tensor_reduce(ctx[:], my2[:], mybir.AxisListType.X, ALU.add)

                # head
                l1t = pp.tile([DH, 48], F32, tag="l1t")
                nc.sync.dma_start(l1t[:], l1t_d.ap())
                l1b = pp.tile([48, 1], F32, tag="l1b")
                nc.sync.dma_start(l1b[:], l1b_d.ap())
                l2t = pp.tile([48, 16], F32, tag="l2t")
                nc.sync.dma_start(l2t[:], l2t_d.ap())
                l2b = pp.tile([16, 1], F32, tag="l2b")
                nc.sync.dma_start(l2b[:], l2b_d.ap())
                hw16 = pp.tile([16, 4], F32, tag="hw16")
                nc.sync.dma_start(hw16[:], hw16_d.ap())
                hw4 = pp.tile([4, 4], F32, tag="hw4")
                nc.sync.dma_start(hw4[:], hw4_d.ap())
                hb = pp.tile([4, 1], F32, tag="hb")
                nc.sync.dma_start(hb[:], hb_d.ap())
                prev = pp.tile([4, 4], F32, tag="prev")
                nc.sync.dma_start(prev[:], prev_d.ap())

                h1p = dps.tile([48, 1], F32, tag="tail_ps")
                nc.tensor.matmul(h1p[0:48, :], l1t[0:DH, :], ctx[0:DH, 0:1],
                                 start=True, stop=True)
                h1 = pp.tile([48, 1], F32, tag="h1")
                nc.scalar.activation(h1[:], h1p[0:48, :], AF.Identity, bias=l1b[:, 0:1])
                h2p = dps.tile([16, 1], F32, tag="tail_ps")
                nc.tensor.matmul(h2p[0:16, :], l2t[0:48, :], h1[0:48, 0:1],
                                 start=True, stop=True)
                h2 = pp.tile([16, 1], F32, tag="h2")
                nc.scalar.activation(h2[:], h2p[0:16, :], AF.Identity, bias=l2b[:, 0:1])
                op_ = dps.tile([4, 1], F32, tag="tail_ps")
                nc.tensor.matmul(op_[0:4, :], hw16[0:16, :], h2[0:16, 0:1],
                                 start=True, stop=True)
                pv = pp.tile([4, 4], F32, tag="pv")
                nc.vector.tensor_mul(pv[:], prev[:], hw4[:])
                pvs = pp.tile([4, 1], F32, tag="pvs")
                nc.vector.tensor_reduce(pvs[:], pv[:], mybir.AxisListType.X, ALU.add)
                r1 = pp.tile([4, 1], F32, tag="r1")
                nc.vector.tensor_add(r1[:], op_[0:4, :], pvs[:])
                res_sb = pp.tile([4, 1], F32, tag="res_sb")
                nc.vector.tensor_add(res_sb[:], r1[:], hb[:])
                nc.sync.dma_start(res_d.ap(), res_sb[:])
            ctxD.__exit__(None, None, None)

    nc.compile()
    return nc


PERM_H = np.arange(4 * H)                              # gate order i,f,g,o kept
PERM_G4 = [0, 1, 2, 3]
# per-gate fp8 scale: 16x, g-gate 32x (tanh(g) = 2*sig(2g)-1 trick)
SC_H = np.repeat([WS, WS, 2 * WS, WS], H)              # [4H] col scale, i,f,g,o
SC_G4 = np.array([WS, WS, 2 * WS, WS])


def _q8(w):
    return np.asarray(w, np.float32).astype(F8H)


def _prep(inputs):
    """Host-side sharding + layout prep (topic-major sequence order)."""
    X = np.asarray(inputs["X"], np.float32)
    xf = np.ascontiguousarray(X.transpose(1, 0, 2, 3)).reshape(B, T, E)
    shared = {}
    wih0 = np.zeros((EP, 4 * H), np.float32)
    wih0[:E] = np.asarray(inputs["txt_Wih0"], np.float32)[PERM_H].T
    wih0[E] = np.asarray(inputs["txt_b0"], np.float32)[PERM_H]
    shared["wih0"] = _q8(wih0 * SC_H)
    shared["whh0"] = _q8(
        np.asarray(inputs["txt_Whh0"], np.float32)[PERM_H].T * SC_H)
    shared["ident"] = np.eye(128, dtype=BF)
    shared["identf"] = np.eye(32, dtype=np.float32)
    shared["ones_p"] = np.ones((128, 1), BF)
    shared["ones_f"] = np.ones((1, 128), BF)
    shared["ones6"] = np.ones((1, B), BF)
    shared["ones_f32"] = np.ones((1, 64), np.float32)
    for nm, w in (("t_wih0", "top_Wih0"), ("t_whh0", "top_Whh0"),
                  ("t_wih1", "top_Wih1"), ("t_whh1", "top_Whh1")):
        shared[nm] = _q8(np.asarray(inputs[w], np.float32)[PERM_H].T * SC_H)
    shared["t_b0r"] = (np.asarray(inputs["top_b0"], np.float32)[PERM_H]
                       * SC_H).reshape(1, 4 * H).astype(BF)
    b1p = (np.asarray(inputs["top_b1"], np.float32)[PERM_H] * SC_H)
    shared["t_b1bc"] = np.ascontiguousarray(np.broadcast_to(
        b1p.reshape(8, 128).T[:, :, None], (128, 8, DAYS))).astype(BF)
    shared["w1t"] = np.asarray(inputs["w1_W"], np.float32).T.astype(BF)
    shared["w1b"] = np.ascontiguousarray(
        np.asarray(inputs["w1_b"], np.float32).reshape(2, 128).T)
    # day LSTM layer 0 input weights [K=256, 4, DH], fp8 x16 (g x32)
    wm = np.asarray(inputs["day_Wih0"], np.float32)
    shared["d_wih0"] = _q8(
        np.ascontiguousarray(wm.reshape(4, DH, H)[PERM_G4].transpose(2, 0, 1))
        * SC_G4[None, :, None])
    # fused-layer combined recurrence weights [k=(h0|h1), 4, m=(L0|L1)]
    whh0d = np.asarray(inputs["day_Whh0"], np.float32).reshape(4, DH, DH)[PERM_G4]
    wih1d = np.asarray(inputs["day_Wih1"], np.float32).reshape(4, DH, DH)[PERM_G4]
    whh1d = np.asarray(inputs["day_Whh1"], np.float32).reshape(4, DH, DH)[PERM_G4]
    wc = np.zeros((128, 4, 128), np.float32)
    wc[0:DH, :, 0:DH] = whh0d.transpose(2, 0, 1)
    wc[0:DH, :, DH:128] = wih1d.transpose(2, 0, 1)
    wc[DH:128, :, DH:128] = whh1d.transpose(2, 0, 1)
    shared["d_wc"] = _q8(wc * SC_G4[None, :, None])
    b0d = np.asarray(inputs["day_b0"], np.float32).reshape(4, DH)[PERM_G4]
    b1d = np.asarray(inputs["day_b1"], np.float32).reshape(4, DH)[PERM_G4]
    seed = np.zeros((128, 4, DAYS), np.float32)
    seed[0:DH] = (b0d.T * SC_G4[None, :])[:, :, None]
    seed[DH:128] = (b1d.T * SC_G4[None, :])[:, :, None]
    shared["d_seed"] = seed.astype(BF)
    shared["w2t"] = np.ascontiguousarray(np.asarray(inputs["w2_W"], np.float32).T)
    shared["w2b"] = np.asarray(inputs["w2_b"], np.float32).reshape(DH, 1)
    shared["l1t"] = np.ascontiguousarray(np.asarray(inputs["lin1_W"], np.float32).T)
    shared["l1b"] = np.asarray(inputs["lin1_b"], np.float32).reshape(48, 1)
    shared["l2t"] = np.ascontiguousarray(np.asarray(inputs["lin2_W"], np.float32).T)
    shared["l2b"] = np.asarray(inputs["lin2_b"], np.float32).reshape(16, 1)
    hw = np.asarray(inputs["head_W"], np.float32)
    shared["hw16"] = np.ascontiguousarray(hw[:, :16].T)
    shared["hw4"] = np.ascontiguousarray(hw[:, 16:])
    shared["hb"] = np.asarray(inputs["head_b"], np.float32).reshape(4, 1)
    shared["prev"] = np.asarray(inputs["previous_labels"], np.float32)

    in_maps = []
    for r in range(NC_):
        xr = xf[BC * r:BC * (r + 1)]                    # [75, 128, 300]
        xe = np.zeros((T, EP, BCP), np.float32)
        xe[:, :E, 0:BC] = xr.transpose(1, 2, 0)
        xe[:, E, 0:BC] = 1.0
        # super-chunk layout matching on-chip tiles: [sc, p, t, k, b]
        xp = np.ascontiguousarray(
            xe.reshape(T // 8, 8, 3, 128, BCP)
              .transpose(0, 3, 1, 2, 4)).astype(BF)
        m = dict(shared)
        m["x"] = xp
        in_maps.append(m)
    return in_maps


def kernel(**inputs) -> np.ndarray:
    if "nc" not in _cache:
        _cache["nc"] = build()
    nc = _cache["nc"]
    in_maps = _prep(inputs)
    import os
    trace = bool(os.environ.get("KERNEL_TRACE"))
    res = run_bass_kernel_spmd(nc, in_maps, core_ids=list(range(NC_)),
                               trace=trace)
    _cache["last_results"] = res
    return np.asarray(res.results[0]["res"], np.float32)
